# revision 12
# baseline (speedup 1.0000x reference)
"""DetNet Trainium2 kernel v14: 90-layer MLP recurrence, data-parallel over 8 cores.

Per core (2048 samples), features on partitions, batch on free axis.

Key changes vs v13 (PE was 95% busy at 64 N=512-matmul slots/layer -> 40):
- Selector (einsum-reduce) matmuls 2-way col-tiled into 2 PSUM banks,
  interleaved: ~4 slots/chunk instead of 8. Outputs land at psum
  partitions 64:96 (j 0:16) / 96:128 (j 16:30), evacuated straight
  into cA same-base.
- mm2/mm3 combined into one [124-out] matmul pair (t_tilde rep2 + v):
  2 slots instead of 4.
- mm1 reorganized into 2 full-128-row matmuls per output group:
  cA = [v|1|tHA|tHB], HRcomb = [Hr|t]: 4 slots instead of 4+2.
- Bias b1 folded into mm1 via the constant-1.0 row of cA; b2/b3 folded
  into mm23 via a relu'd bias-carrier row of zB.
- t_tilde update (ttr += delta) and u = ik*ttr on GPSIMD; relu(zp2) on
  DVE; relu(zp1) + S/tH evacuations on ACT. Engine-balanced.
- v accumulation, t rep2->rep4 copy, and t->HRcomb copy via DMA (sync).

Layouts (32-aligned):
  cA     [128,2048] f16: v(0:60)|pad|tH j0-15(64:80)+0|tH j16-29(96:110)+0
  HRcomb [128,2048] f16: Hr(0:30)|pad|t(32:62)|ONE=1.0(63)|0
  trep   [128,2048] f16: t rep4 at rows 0,32,64,96
  ttr    [64,2048]  f32: t_tilde rep2 (rows 0:30, 32:62)
  hhy/pbuf [128,8,2048] f16: HHY[32*j4+k, g, b] = HH[b,k,4g+j4]
Weights packed per layer into one [128, 730] f16 DMA:
  [0:241]   W1A (cA matmul; cols 0:127 zp1, 127:241 zp2 + bias-carrier)
  [241:482] WH  (HRcomb matmul)
  [482:730] W23 (cols 0:124 from zA, 124:248 from zB incl b2/b3 row)
"""
import sys
import numpy as np

sys.path.insert(0, "/opt/trn_rl_repo")

from contextlib import ExitStack

import concourse.bass as bass
import concourse.tile as tile
from concourse import mybir
from concourse.bass_utils import run_bass_kernel_spmd

B = 16384
K = 30
LAYERS = 90
VL = 60
ZL = 240
NCORES = 8
BC = B // NCORES          # 2048
NCHUNK = 4
CH = BC // NCHUNK         # 512

F32 = mybir.dt.float32
F16 = mybir.dt.float16

AO = mybir.AluOpType
RELU = mybir.ActivationFunctionType.Relu
LAST_RESULT = None


def build_kernel(inv_kap):
    nc = bass.Bass()

    hr_in = nc.declare_dram_parameter("HrT", [K, BC], F16, isOutput=False)
    one_in = nc.declare_dram_parameter("ONES", [1, BC], F16, isOutput=False)
    hhy_in = nc.declare_dram_parameter("HHY", [128, 8 * BC], F16, isOutput=False)
    w_in = nc.declare_dram_parameter("WPK", [LAYERS, 128, 730], F16, isOutput=False)
    sel_in = nc.declare_dram_parameter("SELS", [128, 256], F16, isOutput=False)
    out_dram = nc.declare_dram_parameter("OUT", [LAYERS, K, BC], F16, isOutput=True)

    with tile.TileContext(nc) as tc, ExitStack() as ctx:
        persist = ctx.enter_context(tc.tile_pool(name="persist", bufs=1))
        wpool = ctx.enter_context(tc.tile_pool(name="w", bufs=3))
        zpool = ctx.enter_context(tc.tile_pool(name="z", bufs=3))
        spool = ctx.enter_context(tc.tile_pool(name="s", bufs=3))
        upool = ctx.enter_context(tc.tile_pool(name="u", bufs=3))
        ppA = ctx.enter_context(tc.tile_pool(name="ps_a", bufs=2, space="PSUM"))
        ppB = ctx.enter_context(tc.tile_pool(name="ps_b", bufs=2, space="PSUM"))
        ppz1 = ctx.enter_context(tc.tile_pool(name="ps_z1", bufs=1, space="PSUM"))
        ppz2 = ctx.enter_context(tc.tile_pool(name="ps_z2", bufs=1, space="PSUM"))
        ppS = ctx.enter_context(tc.tile_pool(name="ps_s", bufs=2, space="PSUM"))

        # ---- persistent state
        cA = persist.tile([128, BC], F16)
        hrc = persist.tile([128, BC], F16)
        trep = persist.tile([128, BC], F16)
        ttr = persist.tile([64, BC], F32)
        hhy = persist.tile([128, 8, BC], F16)
        pbuf = persist.tile([128, 8, BC], F16)
        sels = persist.tile([128, 256], F16)

        nc.vector.memset(cA[:], 0.0)
        nc.vector.memset(hrc[:], 0.0)
        nc.vector.memset(trep[:], 0.0)
        nc.vector.memset(ttr[:], 0.0)
        nc.gpsimd.dma_start(hrc[63:64, :], one_in[:])
        nc.gpsimd.dma_start(hrc[0:K, :], hr_in[:])
        nc.gpsimd.dma_start(hhy[:].rearrange("p a b -> p (a b)"), hhy_in[:])
        nc.gpsimd.dma_start(sels[:], sel_in[:])

        zAs = [None] * NCHUNK
        zBs = [None] * NCHUNK

        for l in range(LAYERS):
            wt = wpool.tile([128, 730], F16, tag="w")
            nc.sync.dma_start(wt[:], w_in[l])
            w1_1 = wt[:, 0:127]
            w1_2 = wt[:, 127:241]
            wh_1 = wt[:, 241:368]
            wh_2 = wt[:, 368:482]
            w23a = wt[0:127, 482:606]
            w23b = wt[0:114, 606:730]

            ik = float(inv_kap[l])

            def products(c):
                cs = bass.ts(c, CH)
                nc.vector.tensor_tensor(
                    pbuf[:, :, cs],
                    trep[:, cs].unsqueeze(1).broadcast_to((128, 8, CH)),
                    hhy[:, :, cs],
                    op=AO.mult,
                )

            def stage0(c):
                # interleaved col-tiled selector chains -> thpA (j 0:16),
                # thpB (j 16:30); evacuate same-base into cA
                cs = bass.ts(c, CH)
                thpA = ppA.tile([128, CH], F32, tag="thA")
                thpB = ppB.tile([128, CH], F32, tag="thB")
                for g in range(4):
                    nc.tensor.matmul(
                        thpA[64:96, :], sels[:, 32 * g : 32 * g + 32],
                        pbuf[:, g, cs],
                        start=(g == 0), stop=(g == 3), tile_position=(0, 64),
                    )
                    nc.tensor.matmul(
                        thpB[96:128, :], sels[:, 32 * (g + 4) : 32 * (g + 4) + 32],
                        pbuf[:, g + 4, cs],
                        start=(g == 0), stop=(g == 3), tile_position=(0, 96),
                    )
                nc.scalar.copy(cA[64:96, cs], thpA[64:96, :])
                nc.scalar.copy(cA[96:128, cs], thpB[96:128, :])

            def stage1(c):
                cs = bass.ts(c, CH)
                zp1 = ppz1.tile([127, CH], F32, tag="z1")
                zp2 = ppz2.tile([114, CH], F32, tag="z2")
                nc.tensor.matmul(zp1[:], w1_1, cA[:, cs], start=True, stop=False)
                nc.tensor.matmul(zp2[:], w1_2, cA[:, cs], start=True, stop=False)
                nc.tensor.matmul(zp1[:], wh_1, hrc[:, cs], start=False, stop=True)
                nc.tensor.matmul(zp2[:], wh_2, hrc[:, cs], start=False, stop=True)
                zA = zpool.tile([127, CH], F16, tag="zA")
                zB = zpool.tile([114, CH], F16, tag="zB")
                nc.scalar.activation(zA[:], zp1[:], RELU)
                nc.vector.tensor_scalar(zB[:], zp2[:], 0.0, None, op0=AO.max)
                zAs[c] = zA
                zBs[c] = zB

            def stage2(c):
                cs = bass.ts(c, CH)
                zA, zB = zAs[c], zBs[c]
                S = ppS.tile([124, CH], F32, tag="S")
                nc.tensor.matmul(S[:], w23a, zA[:], start=True, stop=False)
                nc.tensor.matmul(S[:], w23b, zB[:], start=False, stop=True)
                Ssb = spool.tile([124, CH], F16, tag="Ssb")
                nc.scalar.copy(Ssb[:], S[:])
                # t_tilde rep2 += delta
                nc.gpsimd.tensor_tensor(
                    ttr[0:62, cs], ttr[0:62, cs], Ssb[0:62, :], op=AO.add)
                u = upool.tile([62, CH], F16, tag="u")
                nc.gpsimd.tensor_scalar_mul(u[:], ttr[0:62, cs], ik)
                nc.vector.tensor_scalar(
                    trep[0:62, cs], u[:], 1.0, -1.0, op0=AO.min, op1=AO.max)
                # v += S.v ; t rep2 -> rep4 ; t -> HRcomb
                nc.gpsimd.dma_start(cA[0:VL, cs], Ssb[64:124, :], accum_op=AO.add)
                nc.sync.dma_start(trep[64:126, cs], trep[0:62, cs])
                nc.sync.dma_start(hrc[32:62, cs], trep[0:K, cs])

            if l > 0:
                products(0)
            for c in range(NCHUNK + 2):
                if c < NCHUNK and l > 0:
                    stage0(c)
                if c + 1 < NCHUNK and l > 0:
                    products(c + 1)
                if 1 <= c < NCHUNK + 1:
                    stage1(c - 1)
                if c >= 2:
                    stage2(c - 2)

            nc.sync.dma_start(out_dram[l], trep[0:K, :])

    _split_waits(nc)
    return nc


def _split_waits(nc, limit=1):
    """This toolchain build only accepts one sem-wait per instruction;
    hoist surplus waits onto same-engine NoOps inserted before the inst."""
    ctr = 0
    for f in nc.m.functions:
        for blk in f.blocks:
            insts = blk.instructions
            if not any(
                i.sync_info and i.sync_info.on_wait and len(i.sync_info.on_wait) > limit
                for i in insts
            ):
                continue
            new = []
            for inst in insts:
                si = inst.sync_info
                if si and si.on_wait and len(si.on_wait) > limit:
                    waits = list(si.on_wait)
                    extra, keep = waits[:-limit], waits[-limit:]
                    for w in extra:
                        ctr += 1
                        n = mybir.InstNoOp(name=f"WSPLIT-{ctr}", ins=[], outs=[])
                        n.engine = inst.engine
                        n.sync_info = mybir.SyncInfo(on_wait=[w], on_update=[])
                        new.append(n)
                    si.on_wait = keep
                new.append(inst)
            blk.instructions = new
    return ctr


def _prep_shared(W1, b1, W2, b2, W3, b3):
    """Pack per-layer weights into WPK [L, 128, 730] f16 (see module doc)."""
    L = W1.shape[0]
    W1T = W1.transpose(0, 2, 1)           # [L, 150, 240]: in-dim first
    WPK = np.zeros((L, 128, 730), np.float32)

    # --- W1A [0:241]: cA matmul. cA row -> W1 input col:
    #   0:60 -> v (30:90); 64:80 -> tH j0-15 (120:136);
    #   96:110 -> tH j16-30 (136:150)
    for z0, z1, csl in ((0, 127, slice(0, 127)), (127, 240, slice(127, 240))):
        blk = WPK[:, :, csl]
        blk[:, 0:60, :] = W1T[:, 30:90, z0:z1]
        blk[:, 64:80, :] = W1T[:, 120:136, z0:z1]
        blk[:, 96:110, :] = W1T[:, 136:150, z0:z1]

    # --- WH [241:482]: HRcomb matmul. rows 0:30 Hr (W1 cols 0:30),
    #     rows 32:62 t (W1 cols 90:120), row 63 = ONE -> b1 + carrier
    for z0, z1, csl in ((0, 127, slice(241, 368)), (127, 240, slice(368, 481))):
        blk = WPK[:, :, csl]
        blk[:, 0:30, :] = W1T[:, 0:30, z0:z1]
        blk[:, 32:62, :] = W1T[:, 90:120, z0:z1]
        blk[:, 63, :] = b1[:, z0:z1]
    WPK[:, 63, 481] = 1.0                 # zp2 bias-carrier (-> zp2[113])

    # --- W23 [482:730]: combined mm2 (t_tilde rep2) + mm3 (v)
    # S rows: 0:30 / 32:62 = delta rep2; 64:124 = v-inc
    W2T = W2.transpose(0, 2, 1)           # [L, 240, 30]
    W3T = W3.transpose(0, 2, 1)           # [L, 240, 60]
    # zA piece (cols 482:606): rows 0:127 = z 0:127
    WPK[:, 0:127, 482 + 0 : 482 + 30] = W2T[:, 0:127]
    WPK[:, 0:127, 482 + 32 : 482 + 62] = W2T[:, 0:127]
    WPK[:, 0:127, 482 + 64 : 482 + 124] = W3T[:, 0:127]
    # zB piece (cols 606:730): rows 0:113 = z 127:240; row 113 = biases
    WPK[:, 0:113, 606 + 0 : 606 + 30] = W2T[:, 127:240]
    WPK[:, 0:113, 606 + 32 : 606 + 62] = W2T[:, 127:240]
    WPK[:, 0:113, 606 + 64 : 606 + 124] = W3T[:, 127:240]
    WPK[:, 113, 606 + 0 : 606 + 30] = b2
    WPK[:, 113, 606 + 32 : 606 + 62] = b2
    WPK[:, 113, 606 + 64 : 606 + 124] = b3

    # --- selectors [128, 256]: col-group A g=0..3 (j=4g+j4 in 0:16),
    #     col-group B g=4..7 (cols j-16). thp row = 64/96 + colpos.
    SELS = np.zeros((128, 256), np.float16)
    for j in range(K):
        g, j4 = j // 4, j % 4
        colpos = j if g < 4 else j - 16
        SELS[32 * j4 : 32 * j4 + K, 32 * g + colpos] = 1.0

    return WPK.astype(np.float16), SELS


def kernel(Hr, HH, W1, b1, W2, b2, W3, b3, kappa):
    Hr = np.asarray(Hr, np.float32)
    HH = np.asarray(HH, np.float32)
    W1 = np.asarray(W1, np.float32)
    b1 = np.asarray(b1, np.float32)
    W2 = np.asarray(W2, np.float32)
    b2 = np.asarray(b2, np.float32)
    W3 = np.asarray(W3, np.float32)
    b3 = np.asarray(b3, np.float32)
    kappa = np.asarray(kappa, np.float32)

    WPK, SELS = _prep_shared(W1, b1, W2, b2, W3, b3)
    inv_kap = (1.0 / np.abs(kappa)).astype(np.float32)

    in_maps = []
    for ci in range(NCORES):
        sl = slice(ci * BC, (ci + 1) * BC)
        HrT = np.ascontiguousarray(Hr[sl].T).astype(np.float16)
        # HHY[32*j4+k, (g, b)] = HH[b, k, 4g+j4] (j pad to 32)
        HHp = np.zeros((BC, K, 32), np.float32)
        HHp[:, :, :K] = HH[sl]
        HHY = np.zeros((128, 8, BC), np.float16)
        for j4 in range(4):
            HHY[32 * j4 : 32 * j4 + K, :, :] = (
                HHp[:, :, j4::4].transpose(1, 2, 0).astype(np.float16))
        in_maps.append({
            "HrT": HrT, "ONES": np.ones((1, BC), np.float16),
            "HHY": np.ascontiguousarray(HHY.reshape(128, 8 * BC)),
            "WPK": WPK, "SELS": SELS,
        })

    nc = build_kernel(inv_kap)
    res = run_bass_kernel_spmd(nc, in_maps, list(range(NCORES)))
    global LAST_RESULT
    LAST_RESULT = res
    out = np.concatenate(
        [r["OUT"].transpose(0, 2, 1) for r in res.results], axis=1
    )
    return np.ascontiguousarray(out.astype(np.float32))


# revision 15
# speedup vs baseline: 2.1211x; 2.1211x over previous
"""DetNet Trainium2 kernel v14: 90-layer MLP recurrence, data-parallel over 8 cores.

Per core (2048 samples), features on partitions, batch on free axis.

Key changes vs v13 (PE was 95% busy at 64 N=512-matmul slots/layer -> 40):
- Selector (einsum-reduce) matmuls 2-way col-tiled into 2 PSUM banks,
  interleaved: ~4 slots/chunk instead of 8. Outputs land at psum
  partitions 64:96 (j 0:16) / 96:128 (j 16:30), evacuated straight
  into cA same-base.
- mm2/mm3 combined into one [124-out] matmul pair (t_tilde rep2 + v):
  2 slots instead of 4.
- mm1 reorganized into 2 full-128-row matmuls per output group:
  cA = [v|1|tHA|tHB], HRcomb = [Hr|t]: 4 slots instead of 4+2.
- Bias b1 folded into mm1 via the constant-1.0 row of cA; b2/b3 folded
  into mm23 via a relu'd bias-carrier row of zB.
- t_tilde update (ttr += delta) and u = ik*ttr on GPSIMD; relu(zp2) on
  DVE; relu(zp1) + S/tH evacuations on ACT. Engine-balanced.
- v accumulation, t rep2->rep4 copy, and t->HRcomb copy via DMA (sync).

Layouts (32-aligned):
  cA     [128,2048] f16: v(0:60)|pad|tH j0-15(64:80)+0|tH j16-29(96:110)+0
  HRcomb [128,2048] f16: Hr(0:30)|pad|t(32:62)|ONE=1.0(63)|0
  trep   [128,2048] f16: t rep4 at rows 0,32,64,96
  ttr    [64,2048]  f32: t_tilde rep2 (rows 0:30, 32:62)
  hhy/pbuf [128,8,2048] f16: HHY[32*j4+k, g, b] = HH[b,k,4g+j4]
Weights packed per layer into one [128, 730] f16 DMA:
  [0:241]   W1A (cA matmul; cols 0:127 zp1, 127:241 zp2 + bias-carrier)
  [241:482] WH  (HRcomb matmul)
  [482:730] W23 (cols 0:124 from zA, 124:248 from zB incl b2/b3 row)
"""
import sys
import numpy as np

sys.path.insert(0, "/opt/trn_rl_repo")

from contextlib import ExitStack

import concourse.bass as bass
import concourse.tile as tile
from concourse import mybir
from concourse.bass_utils import run_bass_kernel_spmd

B = 16384
K = 30
LAYERS = 90
VL = 60
ZL = 240
NCORES = 8
BC = B // NCORES          # 2048
NCHUNK = 4
CH = BC // NCHUNK         # 512

F32 = mybir.dt.float32
F16 = mybir.dt.float16

AO = mybir.AluOpType
RELU = mybir.ActivationFunctionType.Relu
LAST_RESULT = None

def build_kernel(inv_kap):
    nc = bass.Bass()

    hr_in = nc.declare_dram_parameter("HrT", [K, BC], F16, isOutput=False)
    one_in = nc.declare_dram_parameter("ONES", [1, BC], F16, isOutput=False)
    hhy_in = nc.declare_dram_parameter("HHY", [128, 8 * BC], F16, isOutput=False)
    w_in = nc.declare_dram_parameter("WPK", [LAYERS, 128, 730], F16, isOutput=False)
    sel_in = nc.declare_dram_parameter("SELS", [128, 256], F16, isOutput=False)
    out_dram = nc.declare_dram_parameter("OUT", [LAYERS, K, BC], F16, isOutput=True)

    with tile.TileContext(nc) as tc, ExitStack() as ctx:
        persist = ctx.enter_context(tc.tile_pool(name="persist", bufs=1))
        wpool = ctx.enter_context(tc.tile_pool(name="w", bufs=3))
        zpool = ctx.enter_context(tc.tile_pool(name="z", bufs=3))
        spool = ctx.enter_context(tc.tile_pool(name="s", bufs=3))
        upool = ctx.enter_context(tc.tile_pool(name="u", bufs=3))
        ppA = ctx.enter_context(tc.tile_pool(name="ps_a", bufs=2, space="PSUM"))
        ppz = ctx.enter_context(tc.tile_pool(name="ps_z", bufs=2, space="PSUM"))
        ppS = ctx.enter_context(tc.tile_pool(name="ps_s", bufs=2, space="PSUM"))

        # ---- persistent state
        cA = persist.tile([128, BC], F16)
        hrc = persist.tile([128, BC], F16)
        trep = persist.tile([128, BC], F16)
        ttr = persist.tile([64, BC], F32)
        hhy = persist.tile([128, 8, BC], F16)
        pbuf = persist.tile([128, 8, BC], F16)
        sels = persist.tile([128, 256], F16)

        nc.vector.memset(cA[:], 0.0)
        nc.vector.memset(hrc[:], 0.0)
        nc.vector.memset(trep[:], 0.0)
        nc.vector.memset(ttr[:], 0.0)
        nc.gpsimd.dma_start(hrc[63:64, :], one_in[:])
        nc.gpsimd.dma_start(hrc[0:K, :], hr_in[:])
        nc.gpsimd.dma_start(hhy[:].rearrange("p a b -> p (a b)"), hhy_in[:])
        nc.gpsimd.dma_start(sels[:], sel_in[:])

        zAs = [None] * NCHUNK
        zBs = [None] * NCHUNK

        for l in range(LAYERS):
            wt = wpool.tile([128, 730], F16, tag="w")
            nc.sync.dma_start(wt[:], w_in[l])
            w1_1 = wt[:, 0:127]
            w1_2 = wt[:, 127:241]
            wh_1 = wt[:, 241:368]
            wh_2 = wt[:, 368:482]
            w23a = wt[0:127, 482:606]
            w23b = wt[0:114, 606:730]

            ik = float(inv_kap[l])

            def products(c):
                cs = bass.ts(c, CH)
                nc.vector.tensor_tensor(
                    pbuf[:, :, cs],
                    trep[:, cs].unsqueeze(1).broadcast_to((128, 8, CH)),
                    hhy[:, :, cs],
                    op=AO.mult,
                )

            def stage0(c):
                # interleaved col-tiled selector chains, ONE psum bank:
                # j 0:16 at rows 64:96, j 16:30 at rows 96:128; one evac
                cs = bass.ts(c, CH)
                thp = ppA.tile([128, CH], F32, tag="th")
                for g in range(4):
                    nc.tensor.matmul(
                        thp[64:96, :], sels[:, 32 * g : 32 * g + 32],
                        pbuf[:, g, cs],
                        start=(g == 0), stop=(g == 3), tile_position=(0, 64),
                    )
                    nc.tensor.matmul(
                        thp[96:128, :], sels[:, 32 * (g + 4) : 32 * (g + 4) + 32],
                        pbuf[:, g + 4, cs],
                        start=(g == 0), stop=(g == 3), tile_position=(0, 96),
                    )
                nc.scalar.copy(cA[64:128, cs], thp[64:128, :])

            def stage1(c):
                cs = bass.ts(c, CH)
                zp = ppz.tile([127, 2, CH], F32, tag="z12")
                nc.tensor.matmul(zp[:, 0, :], w1_1, cA[:, cs],
                                 start=True, stop=False)
                nc.tensor.matmul(zp[0:114, 1, :], w1_2, cA[:, cs],
                                 start=True, stop=False)
                nc.tensor.matmul(zp[:, 0, :], wh_1, hrc[:, cs],
                                 start=False, stop=True)
                nc.tensor.matmul(zp[0:114, 1, :], wh_2, hrc[:, cs],
                                 start=False, stop=True)
                zAB = zpool.tile([127, 2, CH], F16, tag="zAB")
                nc.scalar.activation(zAB[:], zp[:], RELU)
                zAs[c] = zAB

            def stage2(c):
                cs = bass.ts(c, CH)
                zAB = zAs[c]
                S = ppS.tile([124, CH], F32, tag="S")
                nc.tensor.matmul(S[:], w23a, zAB[:, 0, :], start=True, stop=False)
                nc.tensor.matmul(S[:], w23b, zAB[0:114, 1, :],
                                 start=False, stop=True)
                Ssb = spool.tile([124, CH], F16, tag="Ssb")
                nc.scalar.copy(Ssb[:], S[:])
                # t_tilde rep2 += delta
                nc.gpsimd.tensor_tensor(
                    ttr[0:62, cs], ttr[0:62, cs], Ssb[0:62, :], op=AO.add)
                u = upool.tile([62, CH], F16, tag="u")
                nc.vector.tensor_scalar(u[:], ttr[0:62, cs], ik, None, op0=AO.mult)
                nc.vector.tensor_scalar(
                    trep[0:62, cs], u[:], 1.0, -1.0, op0=AO.min, op1=AO.max)
                # v += S.v ; t rep2 -> rep4 ; t -> HRcomb
                nc.gpsimd.dma_start(cA[0:VL, cs], Ssb[64:124, :], accum_op=AO.add)
                nc.sync.dma_start(trep[64:126, cs], trep[0:62, cs])
                nc.sync.dma_start(hrc[32:62, cs], trep[0:K, cs])

            if l > 0:
                products(0)
            for c in range(NCHUNK + 2):
                if c < NCHUNK and l > 0:
                    stage0(c)
                if c + 1 < NCHUNK and l > 0:
                    products(c + 1)
                if 1 <= c < NCHUNK + 1:
                    stage1(c - 1)
                if c >= 2:
                    stage2(c - 2)

            nc.sync.dma_start(out_dram[l], trep[0:K, :])

    _split_waits(nc)
    return nc


def _split_waits(nc, limit=1):
    """This toolchain build only accepts one sem-wait per instruction;
    hoist surplus waits onto same-engine NoOps inserted before the inst."""
    ctr = 0
    for f in nc.m.functions:
        for blk in f.blocks:
            insts = blk.instructions
            if not any(
                i.sync_info and i.sync_info.on_wait and len(i.sync_info.on_wait) > limit
                for i in insts
            ):
                continue
            new = []
            for inst in insts:
                si = inst.sync_info
                if si and si.on_wait and len(si.on_wait) > limit:
                    waits = list(si.on_wait)
                    extra, keep = waits[:-limit], waits[-limit:]
                    for w in extra:
                        ctr += 1
                        n = mybir.InstNoOp(name=f"WSPLIT-{ctr}", ins=[], outs=[])
                        n.engine = inst.engine
                        n.sync_info = mybir.SyncInfo(on_wait=[w], on_update=[])
                        new.append(n)
                    si.on_wait = keep
                new.append(inst)
            blk.instructions = new
    return ctr


def _prep_shared(W1, b1, W2, b2, W3, b3):
    """Pack per-layer weights into WPK [L, 128, 730] f16 (see module doc)."""
    L = W1.shape[0]
    W1T = W1.transpose(0, 2, 1)           # [L, 150, 240]: in-dim first
    WPK = np.zeros((L, 128, 730), np.float32)

    # --- W1A [0:241]: cA matmul. cA row -> W1 input col:
    #   0:60 -> v (30:90); 64:80 -> tH j0-15 (120:136);
    #   96:110 -> tH j16-30 (136:150)
    for z0, z1, csl in ((0, 127, slice(0, 127)), (127, 240, slice(127, 240))):
        blk = WPK[:, :, csl]
        blk[:, 0:60, :] = W1T[:, 30:90, z0:z1]
        blk[:, 64:80, :] = W1T[:, 120:136, z0:z1]
        blk[:, 96:110, :] = W1T[:, 136:150, z0:z1]

    # --- WH [241:482]: HRcomb matmul. rows 0:30 Hr (W1 cols 0:30),
    #     rows 32:62 t (W1 cols 90:120), row 63 = ONE -> b1 + carrier
    for z0, z1, csl in ((0, 127, slice(241, 368)), (127, 240, slice(368, 481))):
        blk = WPK[:, :, csl]
        blk[:, 0:30, :] = W1T[:, 0:30, z0:z1]
        blk[:, 32:62, :] = W1T[:, 90:120, z0:z1]
        blk[:, 63, :] = b1[:, z0:z1]
    WPK[:, 63, 481] = 1.0                 # zp2 bias-carrier (-> zp2[113])

    # --- W23 [482:730]: combined mm2 (t_tilde rep2) + mm3 (v)
    # S rows: 0:30 / 32:62 = delta rep2; 64:124 = v-inc
    W2T = W2.transpose(0, 2, 1)           # [L, 240, 30]
    W3T = W3.transpose(0, 2, 1)           # [L, 240, 60]
    # zA piece (cols 482:606): rows 0:127 = z 0:127
    WPK[:, 0:127, 482 + 0 : 482 + 30] = W2T[:, 0:127]
    WPK[:, 0:127, 482 + 32 : 482 + 62] = W2T[:, 0:127]
    WPK[:, 0:127, 482 + 64 : 482 + 124] = W3T[:, 0:127]
    # zB piece (cols 606:730): rows 0:113 = z 127:240; row 113 = biases
    WPK[:, 0:113, 606 + 0 : 606 + 30] = W2T[:, 127:240]
    WPK[:, 0:113, 606 + 32 : 606 + 62] = W2T[:, 127:240]
    WPK[:, 0:113, 606 + 64 : 606 + 124] = W3T[:, 127:240]
    WPK[:, 113, 606 + 0 : 606 + 30] = b2
    WPK[:, 113, 606 + 32 : 606 + 62] = b2
    WPK[:, 113, 606 + 64 : 606 + 124] = b3

    # --- selectors [128, 256]: col-group A g=0..3 (j=4g+j4 in 0:16),
    #     col-group B g=4..7 (cols j-16). thp row = 64/96 + colpos.
    SELS = np.zeros((128, 256), np.float16)
    for j in range(K):
        g, j4 = j // 4, j % 4
        colpos = j if g < 4 else j - 16
        SELS[32 * j4 : 32 * j4 + K, 32 * g + colpos] = 1.0

    return WPK.astype(np.float16), SELS


def kernel(Hr, HH, W1, b1, W2, b2, W3, b3, kappa):
    Hr = np.asarray(Hr, np.float32)
    HH = np.asarray(HH, np.float32)
    W1 = np.asarray(W1, np.float32)
    b1 = np.asarray(b1, np.float32)
    W2 = np.asarray(W2, np.float32)
    b2 = np.asarray(b2, np.float32)
    W3 = np.asarray(W3, np.float32)
    b3 = np.asarray(b3, np.float32)
    kappa = np.asarray(kappa, np.float32)

    WPK, SELS = _prep_shared(W1, b1, W2, b2, W3, b3)
    inv_kap = (1.0 / np.abs(kappa)).astype(np.float32)

    in_maps = []
    for ci in range(NCORES):
        sl = slice(ci * BC, (ci + 1) * BC)
        HrT = np.ascontiguousarray(Hr[sl].T).astype(np.float16)
        # HHY[32*j4+k, (g, b)] = HH[b, k, 4g+j4] (j pad to 32)
        HHp = np.zeros((BC, K, 32), np.float32)
        HHp[:, :, :K] = HH[sl]
        HHY = np.zeros((128, 8, BC), np.float16)
        for j4 in range(4):
            HHY[32 * j4 : 32 * j4 + K, :, :] = (
                HHp[:, :, j4::4].transpose(1, 2, 0).astype(np.float16))
        in_maps.append({
            "HrT": HrT, "ONES": np.ones((1, BC), np.float16),
            "HHY": np.ascontiguousarray(HHY.reshape(128, 8 * BC)),
            "WPK": WPK, "SELS": SELS,
        })

    nc = build_kernel(inv_kap)
    res = run_bass_kernel_spmd(nc, in_maps, list(range(NCORES)))
    global LAST_RESULT
    LAST_RESULT = res
    out = np.concatenate(
        [r["OUT"].transpose(0, 2, 1) for r in res.results], axis=1
    )
    return np.ascontiguousarray(out.astype(np.float32))


# revision 16
# speedup vs baseline: 2.1597x; 1.0182x over previous
"""DetNet Trainium2 kernel v14: 90-layer MLP recurrence, data-parallel over 8 cores.

Per core (2048 samples), features on partitions, batch on free axis.

Key changes vs v13 (PE was 95% busy at 64 N=512-matmul slots/layer -> 40):
- Selector (einsum-reduce) matmuls 2-way col-tiled into 2 PSUM banks,
  interleaved: ~4 slots/chunk instead of 8. Outputs land at psum
  partitions 64:96 (j 0:16) / 96:128 (j 16:30), evacuated straight
  into cA same-base.
- mm2/mm3 combined into one [124-out] matmul pair (t_tilde rep2 + v):
  2 slots instead of 4.
- mm1 reorganized into 2 full-128-row matmuls per output group:
  cA = [v|1|tHA|tHB], HRcomb = [Hr|t]: 4 slots instead of 4+2.
- Bias b1 folded into mm1 via the constant-1.0 row of cA; b2/b3 folded
  into mm23 via a relu'd bias-carrier row of zB.
- t_tilde update (ttr += delta) and u = ik*ttr on GPSIMD; relu(zp2) on
  DVE; relu(zp1) + S/tH evacuations on ACT. Engine-balanced.
- v accumulation, t rep2->rep4 copy, and t->HRcomb copy via DMA (sync).

Layouts (32-aligned):
  cA     [128,2048] f16: v(0:60)|pad|tH j0-15(64:80)+0|tH j16-29(96:110)+0
  HRcomb [128,2048] f16: Hr(0:30)|pad|t(32:62)|ONE=1.0(63)|0
  trep   [128,2048] f16: t rep4 at rows 0,32,64,96
  ttr    [64,2048]  f32: t_tilde rep2 (rows 0:30, 32:62)
  hhy/pbuf [128,8,2048] f16: HHY[32*j4+k, g, b] = HH[b,k,4g+j4]
Weights packed per layer into one [128, 730] f16 DMA:
  [0:241]   W1A (cA matmul; cols 0:127 zp1, 127:241 zp2 + bias-carrier)
  [241:482] WH  (HRcomb matmul)
  [482:730] W23 (cols 0:124 from zA, 124:248 from zB incl b2/b3 row)
"""
import sys
import numpy as np

sys.path.insert(0, "/opt/trn_rl_repo")

from contextlib import ExitStack

import concourse.bass as bass
import concourse.tile as tile
from concourse import mybir
from concourse.bass_utils import run_bass_kernel_spmd

B = 16384
K = 30
LAYERS = 90
VL = 60
ZL = 240
NCORES = 8
BC = B // NCORES          # 2048
NCHUNK = 4
CH = BC // NCHUNK         # 512

F32 = mybir.dt.float32
F16 = mybir.dt.float16

AO = mybir.AluOpType
RELU = mybir.ActivationFunctionType.Relu
IDENT = mybir.ActivationFunctionType.Identity
LAST_RESULT = None

def build_kernel(inv_kap):
    nc = bass.Bass()

    hr_in = nc.declare_dram_parameter("HrT", [K, BC], F16, isOutput=False)
    one_in = nc.declare_dram_parameter("ONES", [1, BC], F16, isOutput=False)
    hhy_in = nc.declare_dram_parameter("HHY", [128, 8 * BC], F16, isOutput=False)
    w_in = nc.declare_dram_parameter("WPK", [LAYERS, 128, 730], F16, isOutput=False)
    sel_in = nc.declare_dram_parameter("SELS", [128, 256], F16, isOutput=False)
    out_dram = nc.declare_dram_parameter("OUT", [LAYERS, K, BC], F16, isOutput=True)

    with tile.TileContext(nc) as tc, ExitStack() as ctx:
        persist = ctx.enter_context(tc.tile_pool(name="persist", bufs=1))
        wpool = ctx.enter_context(tc.tile_pool(name="w", bufs=3))
        zpool = ctx.enter_context(tc.tile_pool(name="z", bufs=3))
        spool = ctx.enter_context(tc.tile_pool(name="s", bufs=3))
        upool = ctx.enter_context(tc.tile_pool(name="u", bufs=3))
        ppA = ctx.enter_context(tc.tile_pool(name="ps_a", bufs=2, space="PSUM"))
        ppz = ctx.enter_context(tc.tile_pool(name="ps_z", bufs=2, space="PSUM"))
        ppS = ctx.enter_context(tc.tile_pool(name="ps_s", bufs=2, space="PSUM"))

        # ---- persistent state
        cA = persist.tile([128, BC], F16)
        hrc = persist.tile([128, BC], F16)
        trep = persist.tile([128, BC], F16)
        ttr = persist.tile([64, BC], F32)
        hhy = persist.tile([128, 8, BC], F16)
        pbuf = persist.tile([128, 8, BC], F16)
        sels = persist.tile([128, 256], F16)

        nc.vector.memset(cA[:], 0.0)
        nc.vector.memset(hrc[:], 0.0)
        nc.vector.memset(trep[:], 0.0)
        nc.vector.memset(ttr[:], 0.0)
        nc.gpsimd.dma_start(hrc[63:64, :], one_in[:])
        nc.gpsimd.dma_start(hrc[0:K, :], hr_in[:])
        nc.gpsimd.dma_start(hhy[:].rearrange("p a b -> p (a b)"), hhy_in[:])
        nc.gpsimd.dma_start(sels[:], sel_in[:])

        zAs = [None] * NCHUNK
        zBs = [None] * NCHUNK

        for l in range(LAYERS):
            wt = wpool.tile([128, 730], F16, tag="w")
            nc.sync.dma_start(wt[:], w_in[l])
            w1_1 = wt[:, 0:127]
            w1_2 = wt[:, 127:241]
            wh_1 = wt[:, 241:368]
            wh_2 = wt[:, 368:482]
            w23a = wt[0:127, 482:606]
            w23b = wt[0:114, 606:730]

            ik = float(inv_kap[l])

            def products(c):
                cs = bass.ts(c, CH)
                nc.vector.tensor_tensor(
                    pbuf[:, :, cs],
                    trep[:, cs].unsqueeze(1).broadcast_to((128, 8, CH)),
                    hhy[:, :, cs],
                    op=AO.mult,
                )

            def stage0(c):
                # interleaved col-tiled selector chains, ONE psum bank:
                # j 0:16 at rows 64:96, j 16:30 at rows 96:128; one evac
                cs = bass.ts(c, CH)
                thp = ppA.tile([128, CH], F32, tag="th")
                for g in range(4):
                    nc.tensor.matmul(
                        thp[64:96, :], sels[:, 32 * g : 32 * g + 32],
                        pbuf[:, g, cs],
                        start=(g == 0), stop=(g == 3), tile_position=(0, 64),
                    )
                    nc.tensor.matmul(
                        thp[96:128, :], sels[:, 32 * (g + 4) : 32 * (g + 4) + 32],
                        pbuf[:, g + 4, cs],
                        start=(g == 0), stop=(g == 3), tile_position=(0, 96),
                    )
                nc.vector.tensor_copy(cA[64:128, cs], thp[64:128, :])

            def stage1(c):
                cs = bass.ts(c, CH)
                zp = ppz.tile([127, 2, CH], F32, tag="z12")
                nc.tensor.matmul(zp[:, 0, :], w1_1, cA[:, cs],
                                 start=True, stop=False)
                nc.tensor.matmul(zp[0:114, 1, :], w1_2, cA[:, cs],
                                 start=True, stop=False)
                nc.tensor.matmul(zp[:, 0, :], wh_1, hrc[:, cs],
                                 start=False, stop=True)
                nc.tensor.matmul(zp[0:114, 1, :], wh_2, hrc[:, cs],
                                 start=False, stop=True)
                zAB = zpool.tile([127, 2, CH], F16, tag="zAB")
                nc.scalar.activation(zAB[:], zp[:], RELU)
                zAs[c] = zAB

            def stage2(c):
                cs = bass.ts(c, CH)
                zAB = zAs[c]
                S = ppS.tile([124, CH], F32, tag="S")
                nc.tensor.matmul(S[:], w23a, zAB[:, 0, :], start=True, stop=False)
                nc.tensor.matmul(S[:], w23b, zAB[0:114, 1, :],
                                 start=False, stop=True)
                Ssb = spool.tile([124, CH], F16, tag="Ssb")
                nc.scalar.copy(Ssb[:], S[:])
                # t_tilde rep2 += delta
                nc.gpsimd.tensor_tensor(
                    ttr[0:62, cs], ttr[0:62, cs], Ssb[0:62, :], op=AO.add)
                u = upool.tile([62, CH], F16, tag="u")
                nc.scalar.activation(u[:], ttr[0:62, cs], IDENT, scale=ik)
                nc.vector.tensor_scalar(
                    trep[0:62, cs], u[:], 1.0, -1.0, op0=AO.min, op1=AO.max)
                # v += S.v ; t rep2 -> rep4 ; t -> HRcomb
                nc.gpsimd.dma_start(cA[0:VL, cs], Ssb[64:124, :], accum_op=AO.add)
                nc.sync.dma_start(trep[64:126, cs], trep[0:62, cs])
                nc.sync.dma_start(hrc[32:62, cs], trep[0:K, cs])
                # cross-layer pipeline: products for the NEXT layer, one
                # chunk behind (so the rep-copy DMA has a step to land)
                if l + 1 < LAYERS and c >= 1:
                    products(c - 1)

            for c in range(NCHUNK + 2):
                if c < NCHUNK and l > 0:
                    stage0(c)
                if 1 <= c < NCHUNK + 1:
                    stage1(c - 1)
                if c >= 2:
                    stage2(c - 2)

            nc.sync.dma_start(out_dram[l], trep[0:K, :])
            if l + 1 < LAYERS:
                products(NCHUNK - 1)

    _split_waits(nc)
    return nc


def _split_waits(nc, limit=1):
    """This toolchain build only accepts one sem-wait per instruction;
    hoist surplus waits onto same-engine NoOps inserted before the inst."""
    ctr = 0
    for f in nc.m.functions:
        for blk in f.blocks:
            insts = blk.instructions
            if not any(
                i.sync_info and i.sync_info.on_wait and len(i.sync_info.on_wait) > limit
                for i in insts
            ):
                continue
            new = []
            for inst in insts:
                si = inst.sync_info
                if si and si.on_wait and len(si.on_wait) > limit:
                    waits = list(si.on_wait)
                    extra, keep = waits[:-limit], waits[-limit:]
                    for w in extra:
                        ctr += 1
                        n = mybir.InstNoOp(name=f"WSPLIT-{ctr}", ins=[], outs=[])
                        n.engine = inst.engine
                        n.sync_info = mybir.SyncInfo(on_wait=[w], on_update=[])
                        new.append(n)
                    si.on_wait = keep
                new.append(inst)
            blk.instructions = new
    return ctr


def _prep_shared(W1, b1, W2, b2, W3, b3):
    """Pack per-layer weights into WPK [L, 128, 730] f16 (see module doc)."""
    L = W1.shape[0]
    W1T = W1.transpose(0, 2, 1)           # [L, 150, 240]: in-dim first
    WPK = np.zeros((L, 128, 730), np.float32)

    # --- W1A [0:241]: cA matmul. cA row -> W1 input col:
    #   0:60 -> v (30:90); 64:80 -> tH j0-15 (120:136);
    #   96:110 -> tH j16-30 (136:150)
    for z0, z1, csl in ((0, 127, slice(0, 127)), (127, 240, slice(127, 240))):
        blk = WPK[:, :, csl]
        blk[:, 0:60, :] = W1T[:, 30:90, z0:z1]
        blk[:, 64:80, :] = W1T[:, 120:136, z0:z1]
        blk[:, 96:110, :] = W1T[:, 136:150, z0:z1]

    # --- WH [241:482]: HRcomb matmul. rows 0:30 Hr (W1 cols 0:30),
    #     rows 32:62 t (W1 cols 90:120), row 63 = ONE -> b1 + carrier
    for z0, z1, csl in ((0, 127, slice(241, 368)), (127, 240, slice(368, 481))):
        blk = WPK[:, :, csl]
        blk[:, 0:30, :] = W1T[:, 0:30, z0:z1]
        blk[:, 32:62, :] = W1T[:, 90:120, z0:z1]
        blk[:, 63, :] = b1[:, z0:z1]
    WPK[:, 63, 481] = 1.0                 # zp2 bias-carrier (-> zp2[113])

    # --- W23 [482:730]: combined mm2 (t_tilde rep2) + mm3 (v)
    # S rows: 0:30 / 32:62 = delta rep2; 64:124 = v-inc
    W2T = W2.transpose(0, 2, 1)           # [L, 240, 30]
    W3T = W3.transpose(0, 2, 1)           # [L, 240, 60]
    # zA piece (cols 482:606): rows 0:127 = z 0:127
    WPK[:, 0:127, 482 + 0 : 482 + 30] = W2T[:, 0:127]
    WPK[:, 0:127, 482 + 32 : 482 + 62] = W2T[:, 0:127]
    WPK[:, 0:127, 482 + 64 : 482 + 124] = W3T[:, 0:127]
    # zB piece (cols 606:730): rows 0:113 = z 127:240; row 113 = biases
    WPK[:, 0:113, 606 + 0 : 606 + 30] = W2T[:, 127:240]
    WPK[:, 0:113, 606 + 32 : 606 + 62] = W2T[:, 127:240]
    WPK[:, 0:113, 606 + 64 : 606 + 124] = W3T[:, 127:240]
    WPK[:, 113, 606 + 0 : 606 + 30] = b2
    WPK[:, 113, 606 + 32 : 606 + 62] = b2
    WPK[:, 113, 606 + 64 : 606 + 124] = b3

    # --- selectors [128, 256]: col-group A g=0..3 (j=4g+j4 in 0:16),
    #     col-group B g=4..7 (cols j-16). thp row = 64/96 + colpos.
    SELS = np.zeros((128, 256), np.float16)
    for j in range(K):
        g, j4 = j // 4, j % 4
        colpos = j if g < 4 else j - 16
        SELS[32 * j4 : 32 * j4 + K, 32 * g + colpos] = 1.0

    return WPK.astype(np.float16), SELS


def kernel(Hr, HH, W1, b1, W2, b2, W3, b3, kappa):
    Hr = np.asarray(Hr, np.float32)
    HH = np.asarray(HH, np.float32)
    W1 = np.asarray(W1, np.float32)
    b1 = np.asarray(b1, np.float32)
    W2 = np.asarray(W2, np.float32)
    b2 = np.asarray(b2, np.float32)
    W3 = np.asarray(W3, np.float32)
    b3 = np.asarray(b3, np.float32)
    kappa = np.asarray(kappa, np.float32)

    WPK, SELS = _prep_shared(W1, b1, W2, b2, W3, b3)
    inv_kap = (1.0 / np.abs(kappa)).astype(np.float32)

    in_maps = []
    for ci in range(NCORES):
        sl = slice(ci * BC, (ci + 1) * BC)
        HrT = np.ascontiguousarray(Hr[sl].T).astype(np.float16)
        # HHY[32*j4+k, (g, b)] = HH[b, k, 4g+j4] (j pad to 32)
        HHp = np.zeros((BC, K, 32), np.float32)
        HHp[:, :, :K] = HH[sl]
        HHY = np.zeros((128, 8, BC), np.float16)
        for j4 in range(4):
            HHY[32 * j4 : 32 * j4 + K, :, :] = (
                HHp[:, :, j4::4].transpose(1, 2, 0).astype(np.float16))
        in_maps.append({
            "HrT": HrT, "ONES": np.ones((1, BC), np.float16),
            "HHY": np.ascontiguousarray(HHY.reshape(128, 8 * BC)),
            "WPK": WPK, "SELS": SELS,
        })

    nc = build_kernel(inv_kap)
    res = run_bass_kernel_spmd(nc, in_maps, list(range(NCORES)))
    global LAST_RESULT
    LAST_RESULT = res
    out = np.concatenate(
        [r["OUT"].transpose(0, 2, 1) for r in res.results], axis=1
    )
    return np.ascontiguousarray(out.astype(np.float32))
